# revision 60
# baseline (speedup 1.0000x reference)
"""Multi-head attention (B=4, L=2048, E=1024, H=16, D=64) on 8 NeuronCores.

Sharding: batch x head-half. Core c handles batch c//2 and heads
[8*(c%2), 8*(c%2)+8). Each core receives x^T for its batch (4.2MB bf16),
512-column slices of Wq/Wk/Wv, the matching 512-row slice of Wo, and
writes a [L, E] bf16 partial; the host sums the two partials per batch
and adds the bias. This is 4x less I/O than 8-way head-parallel with
full-batch replication.

Per-core schedule: the attention sweep (4 q-tiles x 4 head-pairs x 16
key-chunks) is ScalarE-bound (exp); every other PE phase -- K/V/Q
projections and the output projection -- is emitted as fine-grained
"filler" blocks woven between ST/OT matmuls so the tensor engine never
idles (idle gaps reset its clock-ramp p-state and halve matmul
throughput). OT runs two key-chunks behind
ST/exp so a head-pair's tail never stalls the next head-pair's STs
and boundary STs get run-ahead room. The softmax-normalize broadcast
is split: reciprocal right after the last OT, the rt0-DMA-dependent
broadcast matmul deferred 4 key-chunks into the next head-pair (the
hop's DGE+semaphore latency is ~2.5us). Emission order defines
dependency edges: a filler that READS otn must be emitted after the
deferred norm_b that writes the chunk it reads.

  QT/KT [hd=128, q] = (64*W)^T @ x  in fp8e4 DoubleRow over ec-pairs
        (2x PE rate; the x64 weight scale keeps e4m3 out of subnormals
        and cancels inside softmax via the exp scale)
  ST  [keys=128, q] = KT_h^T @ QT_h  (fp8 DoubleRow, K=2x32: d=ko*32+ki)
  PT  = exp(ST/32/4096) on ScalarE (logits ~N(0,0.1), no max needed)
  OT  [65, q] accum over kc: [ones | V_h]^T @ PT_h (row 64 = denom)
  norm: DVE recip -> DMA row to partition 0 -> bf16 ones-matmul
        broadcast -> DVE mul -> otn bf16
  out [q,E] partial = otn^T @ Wo_slice, accum over 4 hd-chunks,
        written bf16 (halves output traffic; host sums in fp32)

Matmul operands must sit at partitions 0..63/0..127 (base-partition-64
operands wedge this walrus build), hence the shift DMAs re-laying Q/K
from the projection's [hd=128, q] tiles into [32, 2, h, q].
"""

import os
from collections import deque

os.environ.setdefault("NEURON_RT_RESET_CORES", "1")

import numpy as np
import ml_dtypes

import concourse.bass as bass
import concourse.tile as tile
from concourse import mybir
from concourse.bass_utils import run_bass_kernel_spmd

B, L, E = 4, 2048, 1024
H, D = 16, 64
N_CORES = 8
H_LOC = H // 2                # 8 heads per core
HD = H_LOC * D                # 512 hd columns per core
P = 128
QC = 512                      # q/key tile (free dim)
N_QC = L // QC                # 4
N_KC = L // P                 # 16 key chunks of 128
N_EC = E // P                 # 8 contraction chunks for projections
N_CH = HD // P                # 4 hd chunks of 128 (2 heads each)
SCALE = 1.0 / 32.0            # 1/sqrt(E)

ST_FP8 = True                 # ST matmul in fp8e4 + DoubleRow (2x PE rate)
QKPROJ_FP8 = True             # Q/K projections fp8e4 + DoubleRow; weights
                              # pre-scaled x64 on host (dodges e4m3
                              # subnormals), folded back in the exp scale

BF16 = mybir.dt.bfloat16
FP8 = mybir.dt.float8e4
F32 = mybir.dt.float32
QK_DT = FP8 if ST_FP8 else BF16
W_DT = FP8 if QKPROJ_FP8 else BF16
WSCALE = 64.0 if QKPROJ_FP8 else 1.0
SCALE_EXP = SCALE / (WSCALE * WSCALE)

# The walrus in this environment rejects instructions carrying more than one
# semaphore wait condition ("Too many sync wait commands" in setupSyncWait).
# Split the excess onto preceding same-engine InstNoOps: the nops execute in
# order on the engine's sequencer, so blocking semantics are preserved.
MAX_WAITS = 1


def _split_excess_waits(nc, max_waits=MAX_WAITS):
    for bb in nc.main_func.blocks:
        out, changed = [], False
        for ins in bb.instructions:
            si = ins.sync_info
            if si is not None and len(si.on_wait) > max_waits:
                waits = list(si.on_wait)
                head, rest = waits[:-max_waits], waits[-max_waits:]
                k = 0
                while head:
                    chunk, head = head[:max_waits], head[max_waits:]
                    out.append(mybir.InstNoOp(
                        name=f"{ins.name}_wsplit{k}", engine=ins.engine,
                        sync_info=mybir.SyncInfo(on_wait=chunk, on_update=[])))
                    k += 1
                ins.sync_info = mybir.SyncInfo(
                    on_wait=rest, on_update=list(si.on_update))
                changed = True
            out.append(ins)
        if changed:
            bb.instructions = out


def build_nc(split=True):
    nc = bass.Bass()
    xT = nc.dram_tensor("xT", [E, L], BF16, kind="ExternalInput")
    wq = nc.dram_tensor("wq", [E, HD], W_DT, kind="ExternalInput")
    wk = nc.dram_tensor("wk", [E, HD], W_DT, kind="ExternalInput")
    wv = nc.dram_tensor("wv", [E, HD], BF16, kind="ExternalInput")
    wo = nc.dram_tensor("wo", [HD, E], BF16, kind="ExternalInput")
    out = nc.dram_tensor("out", [L, E], BF16, kind="ExternalOutput")

    with tile.TileContext(nc) as tc:
        with (
            tc.tile_pool(name="consts", bufs=1) as consts,
            tc.tile_pool(name="qk", bufs=1) as qkp,       # kt + qt (all qc)
            tc.tile_pool(name="qktmp", bufs=3) as qktmp,
            tc.tile_pool(name="vp", bufs=1) as vp,
            tc.tile_pool(name="ptp", bufs=4) as ptp,
            tc.tile_pool(name="otnp", bufs=1) as otnp,
            tc.tile_pool(name="normp", bufs=2) as normp,
            tc.tile_pool(name="outp", bufs=3) as outp,
            tc.tile_pool(name="psb", bufs=2, space="PSUM") as psb,
            tc.tile_pool(name="psot", bufs=2, space="PSUM") as psot,
        ):
            ones_aux = consts.tile([1, 64], BF16, tag="ones")
            nc.vector.memset(ones_aux[:], 1.0)
            wq_sb = consts.tile([P, N_EC, HD], W_DT, tag="wq")
            wk_sb = consts.tile([P, N_EC, HD], W_DT, tag="wk")
            wv_sb = consts.tile([P, N_EC, HD], BF16, tag="wv")
            wo_sb = consts.tile([P, N_CH, E], BF16, tag="wo")
            # x^T resident for the whole kernel: [e=128, ec, l]; loaded in
            # 512-column blocks, interleaved with the weight loads in
            # first-use order so the first projection starts early.
            xt = consts.tile([P, N_EC, L], BF16, tag="xt")
            if QKPROJ_FP8:
                xt8 = consts.tile([P, N_EC, L], FP8, tag="xt8")
            else:
                xt8 = xt
            xr = xT.rearrange("(o p) l -> p o l", p=P)
            nc.sync.dma_start(wq_sb[:], wq.rearrange("(o p) m -> p o m", p=P))
            nc.scalar.dma_start(xt[:, 0:4, bass.ts(0, QC)],
                                xr[:, 0:4, bass.ts(0, QC)])
            nc.scalar.dma_start(xt[:, 4:8, bass.ts(0, QC)],
                                xr[:, 4:8, bass.ts(0, QC)])
            if QKPROJ_FP8:
                nc.vector.tensor_copy(out=xt8[:, 0:4, bass.ts(0, QC)],
                                      in_=xt[:, 0:4, bass.ts(0, QC)])
                nc.vector.tensor_copy(out=xt8[:, 4:8, bass.ts(0, QC)],
                                      in_=xt[:, 4:8, bass.ts(0, QC)])
            nc.sync.dma_start(wk_sb[:], wk.rearrange("(o p) m -> p o m", p=P))
            nc.sync.dma_start(wv_sb[:], wv.rearrange("(o p) m -> p o m", p=P))
            for cb in range(1, N_QC):
                nc.sync.dma_start(xt[:, :, bass.ts(cb, QC)],
                                  xr[:, :, bass.ts(cb, QC)])
                if QKPROJ_FP8:
                    nc.vector.tensor_copy(out=xt8[:, :, bass.ts(cb, QC)],
                                          in_=xt[:, :, bass.ts(cb, QC)])
            nc.sync.dma_start(wo_sb[:], wo.rearrange("(o p) m -> p o m", p=P))

            # K^T and Q^T in fp8 DoubleRow layout [ki=32, ko=2, h, l],
            # d = ko*32 + ki (bf16 fallback: [d=64, h, l]).
            if ST_FP8:
                kt = qkp.tile([32, 2, H_LOC, L], QK_DT, tag="kt")
                qt = qkp.tile([32, 2, H_LOC, L], QK_DT, tag="qt")
            else:
                kt = qkp.tile([64, H_LOC, L], QK_DT, tag="kt")
                qt = qkp.tile([64, H_LOC, L], QK_DT, tag="qt")

            # V natural [key, kc, h, 65]: per head 64 V cols + a ones col.
            vaug = vp.tile([P, N_KC, H_LOC, 65], BF16, tag="vaug")
            nc.vector.memset(vaug[:, :, :, 64], 1.0)

            # normalized attention output, head-pair chunks on partitions:
            # chunk c holds head 2c (parts 0..63) and 2c+1 (parts 64..127)
            otn = otnp.tile([P, N_CH, L], BF16, tag="otn")

            def emit_proj_tile(dst, w_sb, c0, csl):
                """Project hd-chunks (c0, c0+1) x 512 cols `csl` of Q or K
                and shift-DMA into dst (kt/qt layout)."""
                ps = psb.tile([P, 2, QC], F32, tag="big")
                for half in range(2):
                    if QKPROJ_FP8:
                        # DoubleRow over ec pairs: contraction 2x128
                        for j in range(N_EC // 2):
                            nc.tensor.matmul(
                                ps[:, half],
                                lhsT=w_sb[:, 2 * j:2 * j + 2,
                                          bass.ts(c0 + half, P)],
                                rhs=xt8[:, 2 * j:2 * j + 2, csl],
                                start=(j == 0), stop=(j == N_EC // 2 - 1),
                                perf_mode=mybir.MatmulPerfMode.DoubleRow)
                    else:
                        for ec in range(N_EC):
                            nc.tensor.matmul(
                                ps[:, half],
                                lhsT=w_sb[:, ec, bass.ts(c0 + half, P)],
                                rhs=xt[:, ec, csl],
                                start=(ec == 0), stop=(ec == N_EC - 1))
                tmp = qktmp.tile([P, 2, QC], QK_DT, tag="tmp")
                nc.vector.tensor_copy(out=tmp[:], in_=ps[:])
                # issue the shift DMAs round-robin across sequencers --
                # serial issue on one engine (~650ns each) stalls the
                # STs that consume the freshly laid-out tiles
                issuers = [nc.sync, nc.gpsimd]
                n_dma = 0
                for half in range(2):
                    for hl in range(2):          # head within chunk
                        h = 2 * (c0 + half) + hl
                        if ST_FP8:
                            for ko in range(2):  # d 32-halves
                                issuers[n_dma % 2].dma_start(
                                    dst[:, ko, h, csl],
                                    tmp[64 * hl + 32 * ko:
                                        64 * hl + 32 * ko + 32, half])
                                n_dma += 1
                        else:
                            issuers[n_dma % 2].dma_start(
                                dst[:, h, csl],
                                tmp[64 * hl:64 * hl + 64, half])
                            n_dma += 1

            def emit_v_pair(kc0):
                """V for key chunks kc0, kc0+1."""
                ps = psb.tile([P, 2, QC], F32, tag="big")
                for half in range(2):
                    for ec in range(N_EC):
                        nc.tensor.matmul(
                            ps[:, half],
                            lhsT=xt[:, ec, bass.ts(kc0 + half, P)],
                            rhs=wv_sb[:, ec],
                            start=(ec == 0), stop=(ec == N_EC - 1))
                for half in range(2):
                    nc.vector.tensor_copy(
                        out=vaug[:, kc0 + half, :, 0:64],
                        in_=ps[:, half].rearrange("p (h d) -> p h d", d=D))

            def emit_v_single(kc):
                """V for one key chunk (finer PE-filler granule)."""
                ps = psb.tile([P, 2, QC], F32, tag="big")
                for ec in range(N_EC):
                    nc.tensor.matmul(
                        ps[:, 0],
                        lhsT=xt[:, ec, bass.ts(kc, P)],
                        rhs=wv_sb[:, ec],
                        start=(ec == 0), stop=(ec == N_EC - 1))
                nc.vector.tensor_copy(
                    out=vaug[:, kc, :, 0:64],
                    in_=ps[:, 0].rearrange("p (h d) -> p h d", d=D))

            def emit_out_half(q8, eh):
                """Output projection, queries [128*q8, +128), cols
                [512*eh, +512) -- half-block PE-filler granule. DMA issue
                alternates SP/GpSimd so back-to-back halves don't
                serialize on one descriptor generator."""
                ops = psb.tile([P, 2, QC], F32, tag="big")
                for ch in range(N_CH):
                    nc.tensor.matmul(
                        ops[:, 0],
                        lhsT=otn[:, ch, bass.ts(q8, P)],
                        rhs=wo_sb[:, ch, bass.ts(eh, QC)],
                        start=(ch == 0), stop=(ch == N_CH - 1))
                osb = outp.tile([P, QC], BF16, tag="osb")
                nc.vector.tensor_copy(out=osb[:], in_=ops[:, 0])
                (nc.sync if eh == 0 else nc.gpsimd).dma_start(
                    out[bass.ts(q8, P), bass.ts(eh, QC)], osb[:])

            def emit_out_block(q8):
                for eh in range(2):
                    emit_out_half(q8, eh)

            def st_lhs(h, kc):
                if ST_FP8:
                    return kt[:, :, h, bass.ts(kc, P)]
                return kt[:, h, bass.ts(kc, P)]

            def st_rhs(h, qsl):
                if ST_FP8:
                    return qt[:, :, h, qsl]
                return qt[:, h, qsl]

            PM = mybir.MatmulPerfMode.DoubleRow if ST_FP8 else None

            def norm_a(ot):
                """Reciprocal of the denominator rows + DMA hop of the
                recip row from partition 64 to partition 0 (PE operands
                at base partition 64 wedge this walrus build)."""
                rt_hi = normp.tile([65, 2 * QC], BF16, tag="rt_hi")
                with nc.allow_low_precision(
                        reason="bf16 softmax recip: 0.4% common-mode "
                        "per-query scale"):
                    nc.vector.reciprocal(rt_hi[64:65, 0:QC], ot[64:65, 0])
                    nc.vector.reciprocal(rt_hi[64:65, QC:2 * QC],
                                         ot[64:65, 1])
                rt0 = normp.tile([1, 2 * QC], BF16, tag="rt0")
                nc.sync.dma_start(rt0[:], rt_hi[64:65, :])
                return rt0

            def norm_b(ot, rt0, qc, hp, rps_pool=None):
                """K=1 ones-matmul broadcast of the reciprocal row, then
                DVE muls into otn chunk hp. Deferred a few kc behind
                norm_a so the rt0 DMA is done before PE hits the rps
                matmuls. The tail call passes psot as rps_pool because
                both psb buffers hold in-flight progressive out tiles."""
                qsl = bass.ts(qc, QC)
                if rps_pool is None:
                    rps = psb.tile([P, 2, QC], F32, tag="big")
                else:
                    rps = rps_pool.tile([65, 2, QC], F32, tag="ot")
                for hl in range(2):
                    nc.tensor.matmul(rps[0:64, hl], lhsT=ones_aux[:],
                                     rhs=rt0[0:1, bass.ts(hl, QC)],
                                     start=True, stop=True)
                rsb = normp.tile([64, 2, QC], F32, tag="rsb")
                nc.vector.tensor_copy(out=rsb[:], in_=rps[0:64])
                nc.vector.tensor_mul(out=otn[0:64, hp, qsl],
                                     in0=ot[0:64, 0], in1=rsb[:, 0])
                o2n = normp.tile([64, QC], BF16, tag="o2n")
                nc.vector.tensor_mul(out=o2n[:], in0=ot[0:64, 1],
                                     in1=rsb[:, 1])
                nc.sync.dma_start(otn[64:128, hp, qsl], o2n[:])

            # ---- filler schedule -------------------------------------
            # fill[(qc, hp)] = [(kc, closure)] emitted inside that
            # (qc, hp) attention sweep once the kc loop reaches `kc`.
            # Constraints: KT chunk c ready before (qc0, hp=c//2*...);
            # V kc ready before (qc0, hp0, kc); QT qc ready before qc
            # starts; out q8 only after otn qc complete.
            fill = {(qc, hp): deque() for qc in range(N_QC)
                    for hp in range(N_CH)}

            def ktp(c0, kb):
                return lambda: emit_proj_tile(kt, wk_sb, c0, bass.ts(kb, QC))

            def qtp(c0, q):
                return lambda: emit_proj_tile(qt, wq_sb, c0, bass.ts(q, QC))

            def vpr(kc0):
                return lambda: emit_v_pair(kc0)

            # hp0 of qc0: produce KT chunks 0-1 / V one key-block ahead;
            # QT chunks 2-3 (needed at hp2) moved out of the prologue
            fill[0, 0].extend(
                [(1, ktp(0, 1)), (3, qtp(2, 0))]
                + [(k - 2, (lambda kk=k: emit_v_single(kk)))
                   for k in range(2, 16)]
                + [(5, ktp(0, 2)), (9, ktp(0, 3))])
            fill[0, 0] = deque(sorted(fill[0, 0], key=lambda t: t[0]))
            fill[0, 1].extend([
                (2, ktp(2, 0)), (6, ktp(2, 1)),
                (10, ktp(2, 2)), (14, ktp(2, 3)),
            ])
            # QT for qc+1 late in qc's sweep; out-proj for qc-1 spread out
            for qc in range(N_QC):
                if qc + 1 < N_QC:
                    fill[qc, 2].append((11, qtp(0, qc + 1)))
                    fill[qc, 3].append((11, qtp(2, qc + 1)))
                if qc > 0:
                    for hp in range(N_CH):
                        # out-halves read otn(qc-1) chunk 3, written by
                        # the deferred norm_b at kc==5 -> slots must be >5
                        fill[qc, hp].append(
                            (6, lambda q8=4 * (qc - 1) + hp:
                             emit_out_half(q8, 0)))
                        fill[qc, hp].append(
                            (9, lambda q8=4 * (qc - 1) + hp:
                             emit_out_half(q8, 1)))

            # ---- PE clock-ramp warmup: the first real projections
            # otherwise run at the 1.2GHz p-state (ramp needs 3us of
            # continuous busy). K=1 dummy matmuls on a memset scratch
            # row keep PE busy through the input-DMA/cast window so the
            # prologue starts warm. Result is never read.
            scratch = consts.tile([1, QC], BF16, tag="scratch")
            nc.vector.memset(scratch[:], 0.0)
            wps = psb.tile([P, 2, QC], F32, tag="big")
            for i in range(44):
                nc.tensor.matmul(wps[0:64, i % 2], lhsT=ones_aux[:],
                                 rhs=scratch[:], start=True, stop=True,
                                 skip_group_check=True)

            # ---- prologue: first ST needs only QT chunks 0-1, KT
            # chunks 0-1 @ key-block 0 and V chunk 0 ----------------
            emit_proj_tile(qt, wq_sb, 0, bass.ts(0, QC))
            emit_proj_tile(kt, wk_sb, 0, bass.ts(0, QC))
            emit_v_single(0)
            emit_v_single(1)

            # ---- attention sweep, OT two kc behind so a head-pair's
            # tail never stalls the next head-pair's STs and boundary
            # STs get extra run-ahead room (ptp bufs=4 covers the
            # 3-deep exp->OT distance) --------------------------------
            pend = deque()     # (ot, pt, h0, h1, kc, qc, hp)
            pend_norm = None   # (ot, rt0, qc, hp) awaiting norm_b
            OT_LAG = 2

            def flush_one():
                nonlocal pend_norm
                pot, ppt, ph0, ph1, pkc, pqc, php = pend.popleft()
                nc.tensor.matmul(pot[:, 0], lhsT=vaug[:, pkc, ph0],
                                 rhs=ppt[:, 0],
                                 start=(pkc == 0), stop=(pkc == N_KC - 1),
                                 skip_group_check=True)
                nc.tensor.matmul(pot[:, 1], lhsT=vaug[:, pkc, ph1],
                                 rhs=ppt[:, 1],
                                 start=(pkc == 0), stop=(pkc == N_KC - 1),
                                 skip_group_check=True)
                if pkc == N_KC - 1:
                    pend_norm = (pot, norm_a(pot), pqc, php)

            def flush_ot(all_pending=False):
                while len(pend) > (0 if all_pending else OT_LAG):
                    flush_one()

            for qc in range(N_QC):
                qsl = bass.ts(qc, QC)
                for hp in range(N_CH):
                    h0, h1 = 2 * hp, 2 * hp + 1
                    fq = fill[qc, hp]
                    ot = psot.tile([65, 2, QC], F32, tag="ot")
                    for kc in range(N_KC):
                        st = psb.tile([P, 2, QC], F32, tag="big")
                        nc.tensor.matmul(st[:, 0], lhsT=st_lhs(h0, kc),
                                         rhs=st_rhs(h0, qsl),
                                         start=True, stop=True, perf_mode=PM)
                        nc.tensor.matmul(st[:, 1], lhsT=st_lhs(h1, kc),
                                         rhs=st_rhs(h1, qsl),
                                         start=True, stop=True, perf_mode=PM)
                        pt = ptp.tile([P, 2, QC], BF16, tag="pt")
                        nc.scalar.activation(pt[:], st[:],
                                             mybir.ActivationFunctionType.Exp,
                                             scale=SCALE_EXP)
                        pend.append((ot, pt, h0, h1, kc, qc, hp))
                        flush_ot()
                        if kc == 5 and pend_norm is not None:
                            norm_b(*pend_norm)
                            pend_norm = None
                        if qc == 0 and hp == 0:
                            while fq and fq[0][0] <= kc:
                                fq.popleft()[1]()
                        elif fq and fq[0][0] <= kc:
                            fq.popleft()[1]()
                    while fq:
                        fq.popleft()[1]()
            flush_ot(all_pending=True)
            # tail: start q8=12's out accumulation (chunks 0-2, already
            # normed) under the last norm's recip+rt0-DMA latency; its
            # chunk-3 matmul lands right after norm_b writes otn chunk 3.
            # Pool order matters: t12 holds psb buf A, norm_b's rps takes
            # buf B, t12 finishes on A before any further allocation.
            q12 = 4 * (N_QC - 1)
            tprog = []
            for q8l in range(2):
                tp = psb.tile([P, 2, QC], F32, tag="big")
                for eh in range(2):
                    for ch in range(3):
                        nc.tensor.matmul(
                            tp[:, eh],
                            lhsT=otn[:, ch, bass.ts(q12 + q8l, P)],
                            rhs=wo_sb[:, ch, bass.ts(eh, QC)],
                            start=(ch == 0), stop=False,
                            skip_group_check=True)
                tprog.append(tp)
            norm_b(*pend_norm, rps_pool=psot)
            for q8l in range(2):
                tp = tprog[q8l]
                for eh in range(2):
                    nc.tensor.matmul(
                        tp[:, eh],
                        lhsT=otn[:, 3, bass.ts(q12 + q8l, P)],
                        rhs=wo_sb[:, 3, bass.ts(eh, QC)],
                        start=False, stop=True, skip_group_check=True)
                osbf = outp.tile([P, E], BF16, tag="osbf")
                nc.scalar.copy(out=osbf[:],
                               in_=tp.rearrange("p a b -> p (a b)"))
                (nc.sync if q8l == 0 else nc.gpsimd).dma_start(
                    out[bass.ts(q12 + q8l, P)], osbf[:])
            for q8l in range(2, 4):
                emit_out_block(q12 + q8l)

    if split:
        _split_excess_waits(nc)
    return nc


_NC_CACHE = None


def _get_nc():
    global _NC_CACHE
    if _NC_CACHE is None:
        _NC_CACHE = build_nc()
    return _NC_CACHE


def prepare_inputs(x, Wq, Wk, Wv, Wo):
    """Host-side shard prep: returns the per-core input maps."""
    bf16 = ml_dtypes.bfloat16
    xT = np.ascontiguousarray(
        np.asarray(x, np.float32).transpose(0, 2, 1)).astype(bf16)
    fp8 = ml_dtypes.float8_e4m3
    w = {}
    for name, W in (("wq", Wq), ("wk", Wk), ("wv", Wv)):
        W = np.asarray(W, np.float32)
        qk = QKPROJ_FP8 and name in ("wq", "wk")
        dt_, sc = (fp8, WSCALE) if qk else (bf16, 1.0)
        w[name] = [np.ascontiguousarray(
            W[:, hh * HD:(hh + 1) * HD] * sc).astype(dt_)
            for hh in range(2)]
    Wo = np.asarray(Wo, np.float32)
    w["wo"] = [np.ascontiguousarray(Wo[hh * HD:(hh + 1) * HD]).astype(bf16)
               for hh in range(2)]
    in_maps = []
    for c in range(N_CORES):
        b, hh = c // 2, c % 2
        in_maps.append({
            "xT": xT[b],
            "wq": w["wq"][hh],
            "wk": w["wk"][hh],
            "wv": w["wv"][hh],
            "wo": w["wo"][hh],
        })
    return in_maps


def kernel(x, Wq, Wk, Wv, Wo, bo):
    nc = _get_nc()
    in_maps = prepare_inputs(x, Wq, Wk, Wv, Wo)
    res = run_bass_kernel_spmd(nc, in_maps, list(range(N_CORES)))
    bo32 = np.asarray(bo, np.float32)
    outs = []
    for b in range(B):
        acc = (res.results[2 * b]["out"].astype(np.float32)
               + res.results[2 * b + 1]["out"].astype(np.float32) + bo32)
        outs.append(acc)
    return np.stack(outs).astype(np.float32)
